# revision 3
# baseline (speedup 1.0000x reference)
"""Trainium2 Bass kernel for CGPCoupler gather-multiply-scatter (segment reduce).

Computation (reference):
    out_tilde = x1[:, r1] * x2[:, r2] * cg[None, :]        # [B, K]
    out = zeros([B, out_dim]).at[:, ro].add(out_tilde)

Structure exploited: the index tables produced by the CG coupler consist of
K/32 runs of 32 consecutive indices (32-aligned) with a constant coefficient
per run.  So the whole op is T block-triples:
    out[:, o*32:+32] += c_t * x1[:, a*32:+32] * x2[:, b*32:+32]
which we execute on-device as fused DVE tensor_tensor_reduce ops
(out = (in0*in1)*scale) over a batch-on-partition layout.

Sharding: data-parallel over batch; 8 cores x 256 rows each. Index tables are
baked into the compiled program (they are setup-time constants).
"""

import numpy as np

N_CORES = 8


# ----------------------------------------------------------------- planning
def _extract_triples(r1, r2, ro, cg):
    """Detect 32-run structure; return (a, b, o, c) per 32-block triple or None."""
    K = cg.shape[0]
    if K % 32 != 0:
        return None
    T = K // 32
    lane = np.arange(32, dtype=np.int64)
    for arr in (r1, r2, ro):
        v = arr.astype(np.int64).reshape(T, 32)
        if not np.all(v == v[:, :1] + lane):
            return None
        if np.any(v[:, 0] % 32):
            return None
    cgv = cg.reshape(T, 32)
    if not np.all(cgv == cgv[:, :1]):
        return None
    a = (r1.astype(np.int64)[::32] // 32).astype(int)
    b = (r2.astype(np.int64)[::32] // 32).astype(int)
    o = (ro.astype(np.int64)[::32] // 32).astype(int)
    c = cgv[:, 0].astype(np.float64)
    return a, b, o, c


def _numpy_fallback(x1, x2, cg_tilde, repids_in1, repids_in2, repids_out, out_dim):
    out_tilde = x1[:, repids_in1] * x2[:, repids_in2] * cg_tilde[None, :]
    out = np.zeros((x1.shape[0], int(out_dim)), dtype=x1.dtype)
    np.add.at(out, (slice(None), repids_out), out_tilde)
    return out


# ----------------------------------------------------------------- bass build
_nc_cache = {}


def _build_nc(plan_key, groups, in_dim, out_dim, b_core):
    import concourse.bacc as bacc
    from concourse import mybir
    from concourse.tile import TileContext
    from concourse.dve_ops import TENSOR_TENSOR_REDUCE

    f32 = mybir.dt.float32
    S = b_core // 128
    assert b_core % 128 == 0

    nc = bacc.Bacc("TRN2", target_bir_lowering=False)
    x1 = nc.dram_tensor("x1", [b_core, in_dim], f32, kind="ExternalInput")
    x2 = nc.dram_tensor("x2", [b_core, in_dim], f32, kind="ExternalInput")
    y = nc.dram_tensor("y", [b_core, out_dim], f32, kind="ExternalOutput")

    n_oblk = out_dim // 32
    # store in column chunks for DMA/compute overlap
    n_store_chunks = 8
    chunk_edges = np.linspace(0, n_oblk, n_store_chunks + 1).astype(int)

    with TileContext(nc) as tc:
        with (
            tc.tile_pool(name="pin", bufs=1) as pin,
            tc.tile_pool(name="pout", bufs=1) as pout,
            tc.tile_pool(name="ptmp", bufs=16) as ptmp,
            tc.tile_pool(name="pjunk", bufs=16) as pjunk,
        ):
            x1t = pin.tile([128, S, in_dim], f32, tag="x1t")
            x2t = pin.tile([128, S, in_dim], f32, tag="x2t")
            nc.sync.dma_start(out=x1t[:], in_=x1[:].rearrange("(s p) d -> p s d", p=128))
            nc.sync.dma_start(out=x2t[:], in_=x2[:].rearrange("(s p) d -> p s d", p=128))
            outt = pout.tile([128, S, out_dim], f32, tag="outt")

            mult = mybir.AluOpType.mult
            add = mybir.AluOpType.add

            for ci in range(n_store_chunks):
                o_lo, o_hi = chunk_edges[ci], chunk_edges[ci + 1]
                for o in range(o_lo, o_hi):
                    contribs = groups.get(o)
                    if contribs is None:
                        nc.vector.memset(outt[:, :, o * 32:(o + 1) * 32], 0.0)
                        continue
                    dst = outt[:, :, o * 32:(o + 1) * 32]
                    for idx, (a, b, c) in enumerate(contribs):
                        i0 = x1t[:, :, a * 32:(a + 1) * 32]
                        i1 = x2t[:, :, b * 32:(b + 1) * 32]
                        junk = pjunk.tile([128, 1], f32, tag="junk")
                        if idx == 0:
                            nc.vector._custom_dve(
                                TENSOR_TENSOR_REDUCE, out=dst, in0=i0, in1=i1,
                                s0=0.0, s1=float(c), accum_out=junk[:],
                            )
                        else:
                            tmp = ptmp.tile([128, S, 32], f32, tag="tmp")
                            nc.vector._custom_dve(
                                TENSOR_TENSOR_REDUCE, out=tmp[:], in0=i0, in1=i1,
                                s0=0.0, s1=float(c), accum_out=junk[:],
                            )
                            nc.vector.tensor_add(out=dst, in0=dst, in1=tmp[:])
                # store this chunk
                c_lo, c_hi = o_lo * 32, o_hi * 32
                nc.sync.dma_start(
                    out=y[:, c_lo:c_hi].rearrange("(s p) d -> p s d", p=128),
                    in_=outt[:, :, c_lo:c_hi],
                )
    nc.finalize()
    return nc


def _get_nc(triples, in_dim, out_dim, b_core):
    a, b, o, c = triples
    key = (in_dim, out_dim, b_core, a.tobytes() if hasattr(a, "tobytes") else tuple(a),
           tuple(b), tuple(o), tuple(np.asarray(c).tolist()))
    key = hash(key)
    if key not in _nc_cache:
        groups = {}
        for i in range(len(a)):
            groups.setdefault(int(o[i]), []).append((int(a[i]), int(b[i]), float(c[i])))
        _nc_cache[key] = _build_nc(key, groups, in_dim, out_dim, b_core)
    return _nc_cache[key]


# ----------------------------------------------------------------- entry point
def kernel(x1, x2, cg_tilde, repids_in1, repids_in2, repids_out, out_dim):
    x1 = np.asarray(x1, dtype=np.float32)
    x2 = np.asarray(x2, dtype=np.float32)
    cg = np.asarray(cg_tilde, dtype=np.float32)
    r1 = np.asarray(repids_in1)
    r2 = np.asarray(repids_in2)
    ro = np.asarray(repids_out)
    odim = int(np.asarray(out_dim))

    B, in_dim = x1.shape
    triples = _extract_triples(r1, r2, ro, cg)
    if triples is None or B % N_CORES != 0 or (B // N_CORES) % 128 != 0 \
            or odim % 32 != 0:
        return _numpy_fallback(x1, x2, cg, r1, r2, ro, odim)

    from concourse.bass_utils import run_bass_kernel_spmd

    b_core = B // N_CORES
    nc = _get_nc(triples, in_dim, odim, b_core)

    in_maps = [
        {"x1": x1[i * b_core:(i + 1) * b_core],
         "x2": x2[i * b_core:(i + 1) * b_core]}
        for i in range(N_CORES)
    ]
    res = run_bass_kernel_spmd(nc, in_maps, core_ids=list(range(N_CORES)))
    out = np.empty((B, odim), dtype=np.float32)
    for i in range(N_CORES):
        out[i * b_core:(i + 1) * b_core] = res.results[i]["y"]
    return out


# revision 12
# speedup vs baseline: 1.0066x; 1.0066x over previous
"""Trainium2 Bass kernel for CGPCoupler gather-multiply-scatter (segment reduce).

Computation (reference):
    out_tilde = x1[:, r1] * x2[:, r2] * cg[None, :]        # [B, K]
    out = zeros([B, out_dim]).at[:, ro].add(out_tilde)

Structure exploited: the CG coupler's index tables consist of K/32 runs of 32
consecutive 32-aligned indices with a constant coefficient per run, i.e. T
block-triples:
    out[:, o*32:+32] += c_t * x1[:, a*32:+32] * x2[:, b*32:+32]

Device mapping:
  * data-parallel over batch: 8 cores x 256 rows.
  * SBUF layout: batch-on-partition, 2 batch subtiles packed into the free
    dim so a 32-col block is one contiguous 64-wide column group
    (col = blk*64 + s*32 + c).  HBM stores stay fully contiguous per row.
  * per triple, one fused DVE op (custom TENSOR_TENSOR_REDUCE:
    out = in0*in1*s1).  Triples are grouped into affine chains
    (a0+k*da, b0+k*db, dst0+k*dd) with equal coefficient -> one rank-3
    strided-AP instruction per chain.
  * first contribution per out block writes directly; later contributions
    go to rank-class tmp arrays and are folded in with run-grouped adds.
"""

import numpy as np
from collections import Counter, defaultdict

N_CORES = 8


# ----------------------------------------------------------------- planning
def _extract_triples(r1, r2, ro, cg):
    """Detect 32-run structure; return (a, b, o, c) per 32-block triple or None."""
    K = cg.shape[0]
    if K % 32 != 0:
        return None
    T = K // 32
    lane = np.arange(32, dtype=np.int64)
    for arr in (r1, r2, ro):
        v = arr.astype(np.int64).reshape(T, 32)
        if not np.all(v == v[:, :1] + lane):
            return None
        if np.any(v[:, 0] % 32):
            return None
    cgv = cg.reshape(T, 32)
    if not np.all(cgv == cgv[:, :1]):
        return None
    a = (r1.astype(np.int64)[::32] // 32).astype(int)
    b = (r2.astype(np.int64)[::32] // 32).astype(int)
    o = (ro.astype(np.int64)[::32] // 32).astype(int)
    c = cgv[:, 0].astype(np.float64)
    return a, b, o, c


def _greedy_chains(pts):
    """Cover point set (a,b,dst) by affine chains; returns [(p0, delta, r)]."""
    pts = set(pts)
    groups = []
    while pts:
        pl = sorted(pts)
        if len(pl) == 1:
            groups.append((pl[0], (0, 0, 0), 1))
            pts.remove(pl[0])
            break
        best = None
        for p in pl:
            for q in pl:
                if p >= q:
                    continue
                d = (q[0] - p[0], q[1] - p[1], q[2] - p[2])
                s = p
                while (s[0] - d[0], s[1] - d[1], s[2] - d[2]) in pts:
                    s = (s[0] - d[0], s[1] - d[1], s[2] - d[2])
                chain = [s]
                nxt = (s[0] + d[0], s[1] + d[1], s[2] + d[2])
                while nxt in pts:
                    chain.append(nxt)
                    nxt = (nxt[0] + d[0], nxt[1] + d[1], nxt[2] + d[2])
                if best is None or len(chain) > len(best[0]):
                    best = (chain, d)
        chain, d = best
        if d[2] < 0:  # canonicalize: dst stride positive
            chain = chain[::-1]
            d = (-d[0], -d[1], -d[2])
        groups.append((chain[0], d, len(chain)))
        for p in set(chain):
            pts.discard(p)
    return groups


def _make_plan(a, b, o, c):
    T = len(a)
    order = np.lexsort((np.arange(T), o))
    cnt = Counter()
    rank = np.zeros(T, int)
    for i in order:
        cnt[o[i]] += 1
        rank[i] = cnt[o[i]]
    max_rank = int(rank.max()) if T else 0

    cr = np.round(c, 12)
    ttr_chains = []          # (rank, c, (a0,b0,d0), (da,db,dd), r)
    tmp_sizes = {}
    add_runs = []            # (rank, o0, j0, r)
    for rk in range(1, max_rank + 1):
        idxs = [i for i in range(T) if rank[i] == rk]
        idxs.sort(key=lambda i: o[i])
        if rk > 1:
            tmp_sizes[rk] = len(idxs)
            jof = {i: j for j, i in enumerate(idxs)}
            # add runs: consecutive o (and hence consecutive j)
            start = 0
            for k in range(1, len(idxs) + 1):
                if k == len(idxs) or o[idxs[k]] != o[idxs[k - 1]] + 1:
                    add_runs.append((rk, int(o[idxs[start]]), start, k - start))
                    start = k
        classes = defaultdict(list)
        for i in idxs:
            classes[cr[i]].append(i)
        for cv, ii in classes.items():
            if rk == 1:
                pts = [(int(a[i]), int(b[i]), int(o[i])) for i in ii]
            else:
                pts = [(int(a[i]), int(b[i]), int(jof[i])) for i in ii]
            for p0, d, r in _greedy_chains(pts):
                ttr_chains.append((rk, float(cv), p0, d, r))
    return ttr_chains, tmp_sizes, add_runs


def _numpy_fallback(x1, x2, cg_tilde, repids_in1, repids_in2, repids_out, out_dim):
    out_tilde = x1[:, repids_in1] * x2[:, repids_in2] * cg_tilde[None, :]
    out = np.zeros((x1.shape[0], int(out_dim)), dtype=x1.dtype)
    np.add.at(out, (slice(None), repids_out), out_tilde)
    return out


# ----------------------------------------------------------------- bass build
_nc_cache = {}


def _slice_blocks(ap, start, step, r, P=128):
    """[P, nblk, 64] AP -> [P, r, 64] starting at `start` with block stride `step`."""
    if r == 1:
        return ap[:, start:start + 1, :]
    if step == 0:
        return ap[:, start:start + 1, :].to_broadcast([P, r, 64])
    if step > 0:
        return ap[:, start: start + step * (r - 1) + 1: step, :]
    stop = start + step * (r - 1) - 1
    return ap[:, start: (stop if stop >= 0 else None): step, :]


def _build_nc(ttr_chains, tmp_sizes, add_runs, in_dim, out_dim, b_core,
              n_store_chunks=8, no_broadcast=False):
    import concourse.bacc as bacc
    from concourse import mybir
    from concourse.tile import TileContext
    from concourse.dve_ops import TENSOR_TENSOR_REDUCE

    f32 = mybir.dt.float32
    S = b_core // 128
    assert S == 2, "layout assumes 2 batch subtiles per core"
    n_ablk = in_dim // 32
    n_oblk = out_dim // 32

    nc = bacc.Bacc("TRN2", target_bir_lowering=False)
    x1 = nc.dram_tensor("x1", [b_core, in_dim], f32, kind="ExternalInput")
    x2 = nc.dram_tensor("x2", [b_core, in_dim], f32, kind="ExternalInput")
    y = nc.dram_tensor("y", [b_core, out_dim], f32, kind="ExternalOutput")

    chunk_edges = np.linspace(0, n_oblk, n_store_chunks + 1).astype(int)

    with TileContext(nc) as tc:
        with (
            tc.tile_pool(name="pin", bufs=1) as pin,
            tc.tile_pool(name="pout", bufs=1) as pout,
            tc.tile_pool(name="ptmp", bufs=1) as ptmp,
            tc.tile_pool(name="pjunk", bufs=16) as pjunk,
        ):
            x1t = pin.tile([128, n_ablk * 64], f32, tag="x1t")
            x2t = pin.tile([128, n_ablk * 64], f32, tag="x2t")
            # HBM row s*128+p, col f*32+c  ->  SBUF col f*64 + s*32 + c
            # (one DMA per subtile keeps both sides at <=3 AP dims)
            for xt, xd in ((x1t, x1), (x2t, x2)):
                xv = xt[:].rearrange("p (f s c) -> p s f c", s=S, c=32)
                for s in range(S):
                    nc.sync.dma_start(out=xv[:, s], in_=xd[s * 128:(s + 1) * 128, :])
            outt = pout.tile([128, n_oblk * 64], f32, tag="outt")

            x1b = x1t[:].rearrange("p (f v) -> p f v", v=64)
            x2b = x2t[:].rearrange("p (f v) -> p f v", v=64)
            outb = outt[:].rearrange("p (f v) -> p f v", v=64)
            tmps = {}
            tmps_flat = {}
            for rk, sz in tmp_sizes.items():
                t = ptmp.tile([128, sz * 64], f32, tag=f"tmp{rk}")
                tmps_flat[rk] = t
                tmps[rk] = t[:].rearrange("p (f v) -> p f v", v=64)

            # rank>=2 chains first (independent of outt), then rank-1, then adds
            for want_rk1 in (False, True):
                for rk, cv, p0, d, r in ttr_chains:
                    if (rk == 1) != want_rk1:
                        continue
                    a0, b0, d0 = p0
                    da, db, dd = d
                    dstb = outb if rk == 1 else tmps[rk]
                    pieces = [(a0, b0, d0, da, db, dd, r)]
                    if no_broadcast and r > 1:
                        # interp-only mode: the CoreSim custom-DVE reference
                        # can't handle mixed merged/strided AP shapes
                        pieces = [(a0 + k * da, b0 + k * db, d0 + k * dd,
                                   0, 0, 0, 1) for k in range(r)]
                    for (ca, cb, cd, xda, xdb, xdd, cr) in pieces:
                        junk = pjunk.tile([128, 1], f32, tag="junk")
                        nc.vector._custom_dve(
                            TENSOR_TENSOR_REDUCE,
                            out=_slice_blocks(dstb, cd, xdd, cr),
                            in0=_slice_blocks(x1b, ca, xda, cr),
                            in1=_slice_blocks(x2b, cb, xdb, cr),
                            s0=0.0, s1=float(cv), accum_out=junk[:],
                        )
            for rk, o0, j0, r in sorted(add_runs, key=lambda t: (t[1], t[0])):
                nc.vector.tensor_add(
                    out=outt[:, o0 * 64:(o0 + r) * 64],
                    in0=outt[:, o0 * 64:(o0 + r) * 64],
                    in1=tmps_flat[rk][:, j0 * 64:(j0 + r) * 64],
                )
            # stores (one DMA per chunk per subtile; HBM side contiguous rows)
            outv = outt[:].rearrange("p (f s c) -> p s f c", s=S, c=32)
            for ci in range(n_store_chunks):
                o_lo, o_hi = int(chunk_edges[ci]), int(chunk_edges[ci + 1])
                if o_hi <= o_lo:
                    continue
                c_lo, c_hi = o_lo * 32, o_hi * 32
                for s in range(S):
                    nc.sync.dma_start(
                        out=y[s * 128:(s + 1) * 128, c_lo:c_hi],
                        in_=outv[:, s, o_lo:o_hi, :],
                    )
    nc.finalize()
    return nc


def _get_nc(triples, in_dim, out_dim, b_core, no_broadcast=False):
    a, b, o, c = triples
    key = hash((in_dim, out_dim, b_core, no_broadcast, tuple(a), tuple(b),
                tuple(o), tuple(np.asarray(c).tolist())))
    if key not in _nc_cache:
        ttr_chains, tmp_sizes, add_runs = _make_plan(a, b, o, c)
        _nc_cache[key] = _build_nc(ttr_chains, tmp_sizes, add_runs,
                                   in_dim, out_dim, b_core,
                                   no_broadcast=no_broadcast)
    return _nc_cache[key]


# ----------------------------------------------------------------- entry point
def kernel(x1, x2, cg_tilde, repids_in1, repids_in2, repids_out, out_dim):
    x1 = np.ascontiguousarray(np.asarray(x1, dtype=np.float32))
    x2 = np.ascontiguousarray(np.asarray(x2, dtype=np.float32))
    cg = np.asarray(cg_tilde, dtype=np.float32)
    r1 = np.asarray(repids_in1)
    r2 = np.asarray(repids_in2)
    ro = np.asarray(repids_out)
    odim = int(np.asarray(out_dim))

    B, in_dim = x1.shape
    triples = _extract_triples(r1, r2, ro, cg)
    usable = (
        triples is not None and B % N_CORES == 0
        and (B // N_CORES) == 256 and odim % 32 == 0 and in_dim % 32 == 0
    )
    if not usable:
        return _numpy_fallback(x1, x2, cg, r1, r2, ro, odim)

    from concourse.bass_utils import run_bass_kernel_spmd

    b_core = B // N_CORES
    nc = _get_nc(triples, in_dim, odim, b_core)

    in_maps = [
        {"x1": x1[i * b_core:(i + 1) * b_core],
         "x2": x2[i * b_core:(i + 1) * b_core]}
        for i in range(N_CORES)
    ]
    res = run_bass_kernel_spmd(nc, in_maps, core_ids=list(range(N_CORES)))
    out = np.empty((B, odim), dtype=np.float32)
    for i in range(N_CORES):
        out[i * b_core:(i + 1) * b_core] = res.results[i]["y"]
    return out


# revision 21
# speedup vs baseline: 1.6500x; 1.6392x over previous
"""Trainium2 Bass kernel for CGPCoupler gather-multiply-scatter (segment reduce).

Computation (reference):
    out_tilde = x1[:, r1] * x2[:, r2] * cg[None, :]        # [B, K]
    out = zeros([B, out_dim]).at[:, ro].add(out_tilde)

Structure exploited: the CG coupler's index tables consist of K/32 runs of 32
consecutive 32-aligned indices with a constant coefficient per run, i.e. T
block-triples:
    out[:, o*32:+32] += c_t * x1[:, a*32:+32] * x2[:, b*32:+32]

Device mapping:
  * data-parallel over batch: 8 cores x 256 rows.
  * SBUF layout: batch-on-partition, 2 batch subtiles packed into the free
    dim so a 32-col block is one contiguous 64-wide column group
    (col = blk*64 + s*32 + c).  HBM stores stay fully contiguous per row.
  * per triple, one fused DVE op (custom TENSOR_TENSOR_REDUCE:
    out = in0*in1*s1).  Triples are grouped into affine chains
    (a0+k*da, b0+k*db, dst0+k*dd) with equal coefficient -> one rank-3
    strided-AP instruction per chain.
  * first contribution per out block writes directly; later contributions
    go to rank-class tmp arrays and are folded in with run-grouped adds.
"""

import numpy as np
from collections import Counter, defaultdict

N_CORES = 8


# ----------------------------------------------------------------- planning
def _extract_triples(r1, r2, ro, cg):
    """Detect 32-run structure; return (a, b, o, c) per 32-block triple or None."""
    K = cg.shape[0]
    if K % 32 != 0:
        return None
    T = K // 32
    lane = np.arange(32, dtype=np.int64)
    for arr in (r1, r2, ro):
        v = arr.astype(np.int64).reshape(T, 32)
        if not np.all(v == v[:, :1] + lane):
            return None
        if np.any(v[:, 0] % 32):
            return None
    cgv = cg.reshape(T, 32)
    if not np.all(cgv == cgv[:, :1]):
        return None
    a = (r1.astype(np.int64)[::32] // 32).astype(int)
    b = (r2.astype(np.int64)[::32] // 32).astype(int)
    o = (ro.astype(np.int64)[::32] // 32).astype(int)
    c = cgv[:, 0].astype(np.float64)
    return a, b, o, c


def _greedy_chains(pts):
    """Cover point set (a,b,dst) by affine chains; returns [(p0, delta, r)]."""
    pts = set(pts)
    groups = []
    while pts:
        pl = sorted(pts)
        if len(pl) == 1:
            groups.append((pl[0], (0, 0, 0), 1))
            pts.remove(pl[0])
            break
        best = None
        for p in pl:
            for q in pl:
                if p >= q:
                    continue
                d = (q[0] - p[0], q[1] - p[1], q[2] - p[2])
                s = p
                while (s[0] - d[0], s[1] - d[1], s[2] - d[2]) in pts:
                    s = (s[0] - d[0], s[1] - d[1], s[2] - d[2])
                chain = [s]
                nxt = (s[0] + d[0], s[1] + d[1], s[2] + d[2])
                while nxt in pts:
                    chain.append(nxt)
                    nxt = (nxt[0] + d[0], nxt[1] + d[1], nxt[2] + d[2])
                if best is None or len(chain) > len(best[0]):
                    best = (chain, d)
        chain, d = best
        if d[2] < 0:  # canonicalize: dst stride positive
            chain = chain[::-1]
            d = (-d[0], -d[1], -d[2])
        groups.append((chain[0], d, len(chain)))
        for p in set(chain):
            pts.discard(p)
    return groups


def _make_plan(a, b, o, c):
    T = len(a)
    order = np.lexsort((np.arange(T), o))
    cnt = Counter()
    rank = np.zeros(T, int)
    for i in order:
        cnt[o[i]] += 1
        rank[i] = cnt[o[i]]
    max_rank = int(rank.max()) if T else 0

    cr = np.round(c, 12)
    ttr_chains = []          # (rank, c, (a0,b0,d0), (da,db,dd), r)
    tmp_sizes = {}
    add_runs = []            # (rank, o0, j0, r)
    for rk in range(1, max_rank + 1):
        idxs = [i for i in range(T) if rank[i] == rk]
        idxs.sort(key=lambda i: o[i])
        if rk > 1:
            tmp_sizes[rk] = len(idxs)
            jof = {i: j for j, i in enumerate(idxs)}
            # add runs: consecutive o (and hence consecutive j)
            start = 0
            for k in range(1, len(idxs) + 1):
                if k == len(idxs) or o[idxs[k]] != o[idxs[k - 1]] + 1:
                    add_runs.append((rk, int(o[idxs[start]]), start, k - start))
                    start = k
        classes = defaultdict(list)
        for i in idxs:
            classes[cr[i]].append(i)
        for cv, ii in classes.items():
            if rk == 1:
                pts = [(int(a[i]), int(b[i]), int(o[i])) for i in ii]
            else:
                pts = [(int(a[i]), int(b[i]), int(jof[i])) for i in ii]
            for p0, d, r in _greedy_chains(pts):
                ttr_chains.append((rk, float(cv), p0, d, r))
    return ttr_chains, tmp_sizes, add_runs


def _numpy_fallback(x1, x2, cg_tilde, repids_in1, repids_in2, repids_out, out_dim):
    out_tilde = x1[:, repids_in1] * x2[:, repids_in2] * cg_tilde[None, :]
    out = np.zeros((x1.shape[0], int(out_dim)), dtype=x1.dtype)
    np.add.at(out, (slice(None), repids_out), out_tilde)
    return out


# ----------------------------------------------------------------- bass build
_nc_cache = {}


def _slice_blocks(ap, start, step, r, P=128):
    """[P, nblk, 64] AP -> [P, r, 64] starting at `start` with block stride `step`."""
    if r == 1:
        return ap[:, start:start + 1, :]
    if step == 0:
        return ap[:, start:start + 1, :].to_broadcast([P, r, 64])
    if step > 0:
        return ap[:, start: start + step * (r - 1) + 1: step, :]
    stop = start + step * (r - 1) - 1
    return ap[:, start: (stop if stop >= 0 else None): step, :]


def _build_nc(ttr_chains, tmp_sizes, add_runs, in_dim, out_dim, b_core,
              n_store_chunks=8, no_broadcast=False):
    import concourse.bacc as bacc
    from concourse import mybir
    from concourse.tile import TileContext
    from concourse.dve_ops import TENSOR_TENSOR_REDUCE

    f32 = mybir.dt.float32
    S = b_core // 128
    assert S == 2, "layout assumes 2 batch subtiles per core"
    n_ablk = in_dim // 32
    n_oblk = out_dim // 32

    nc = bacc.Bacc("TRN2", target_bir_lowering=False)
    x1 = nc.dram_tensor("x1", [b_core, in_dim], f32, kind="ExternalInput")
    x2 = nc.dram_tensor("x2", [b_core, in_dim], f32, kind="ExternalInput")
    y = nc.dram_tensor("y", [b_core, out_dim], f32, kind="ExternalOutput")

    # tapered chunks: big early windows, small tail to shorten the drain
    w = np.array([40, 40, 40, 36, 32, 28, 24, 16, 12, 8, 4], dtype=int)
    w = (w * n_oblk / w.sum()).astype(int)
    w[0] += n_oblk - w.sum()
    chunk_edges = np.concatenate([[0], np.cumsum(w)])
    n_store_chunks = len(w)

    with TileContext(nc) as tc:
        with (
            tc.tile_pool(name="pin", bufs=1) as pin,
            tc.tile_pool(name="pout", bufs=1) as pout,
            tc.tile_pool(name="ptmp", bufs=1) as ptmp,
            tc.tile_pool(name="pstg", bufs=3) as pstg,
            tc.tile_pool(name="pjunk", bufs=32) as pjunk,
        ):
            x1t = pin.tile([128, n_ablk * 64], f32, tag="x1t")
            x2t = pin.tile([128, n_ablk * 64], f32, tag="x2t")
            # contiguous loads into s-major staging, ACT reorders into the
            # blocked layout (SBUF col f*64 + s*32 + c <- HBM row s*128+p,
            # col f*32+c)
            copyf = mybir.ActivationFunctionType.Copy
            for xt, xd, nm in ((x1t, x1, "l1"), (x2t, x2, "l2")):
                lstg = pstg.tile([128, S, in_dim], f32, tag="stg")
                xv = xt[:].rearrange("p (f s c) -> p s f c", s=S, c=32)
                for s in range(S):
                    nc.sync.dma_start(out=lstg[:, s, :],
                                      in_=xd[s * 128:(s + 1) * 128, :])
                    nc.scalar.activation(
                        out=xv[:, s],
                        in_=lstg[:, s, :].rearrange("p (f c) -> p f c", c=32),
                        func=copyf,
                    )
            outt = pout.tile([128, n_oblk * 64], f32, tag="outt")

            x1b = x1t[:].rearrange("p (f v) -> p f v", v=64)
            x2b = x2t[:].rearrange("p (f v) -> p f v", v=64)
            outb = outt[:].rearrange("p (f v) -> p f v", v=64)
            tmps = {}
            tmps_flat = {}
            for rk, sz in tmp_sizes.items():
                t = ptmp.tile([128, sz * 64], f32, tag=f"tmp{rk}")
                tmps_flat[rk] = t
                tmps[rk] = t[:].rearrange("p (f v) -> p f v", v=64)

            # Order compute ops by the MIN output block they touch, then emit
            # each store window as soon as the last op touching it has been
            # emitted.  (Ordering by min keeps long-span chains from delaying
            # early windows; correctness only needs every touching op to
            # precede the window's reorder/store.)
            o_of_tmp = {}
            for rk, o0, j0, r in add_runs:
                for k in range(r):
                    o_of_tmp[(rk, j0 + k)] = o0 + k
            work = []  # (minkey, seq, kind, payload, touched_blocks)
            seq = 0
            for rk, cv, p0, d, r in ttr_chains:
                dsts = [p0[2] + d[2] * k for k in range(r)]
                if rk == 1:
                    touched = dsts
                else:
                    touched = [o_of_tmp[(rk, j)] for j in dsts]
                work.append((min(touched), seq, "ttr", (rk, cv, p0, d, r),
                             touched))
                seq += 1
            for rk, o0, j0, r in add_runs:
                touched = list(range(o0, o0 + r))
                # keyed by MAX block: every producer chain of a block k in the
                # run has key = min(chain blocks) <= k <= max and earlier seq,
                # so all tmp/rank-1 writes precede this add
                work.append((max(touched), seq, "add", (rk, o0, j0, r), touched))
                seq += 1
            work.sort(key=lambda t: (t[0], t[1]))
            # window-close position: index of last op touching each window
            close_at = {}
            for idx, (_, _, kind, pl, touched) in enumerate(work):
                for ci in range(n_store_chunks):
                    o_lo, o_hi = int(chunk_edges[ci]), int(chunk_edges[ci + 1])
                    if any(o_lo <= t < o_hi for t in touched):
                        close_at[ci] = idx
            fixed = []
            for idx, (key, sq, kind, pl, _) in enumerate(work):
                fixed.append((kind, pl))
                for ci in range(n_store_chunks):
                    if close_at.get(ci) == idx:
                        o_lo, o_hi = int(chunk_edges[ci]), int(chunk_edges[ci + 1])
                        if o_hi > o_lo:
                            fixed.append(("store", (o_lo, o_hi)))

            outv = outt[:].rearrange("p (f s c) -> p s f c", s=S, c=32)
            for kind, pl in fixed:
                if kind == "ttr":
                    rk, cv, p0, d, r = pl
                    a0, b0, d0 = p0
                    da, db, dd = d
                    dstb = outb if rk == 1 else tmps[rk]
                    pieces = [(a0, b0, d0, da, db, dd, r)]
                    if no_broadcast and r > 1:
                        # interp-only mode: the CoreSim custom-DVE reference
                        # can't handle mixed merged/strided AP shapes
                        pieces = [(a0 + k * da, b0 + k * db, d0 + k * dd,
                                   0, 0, 0, 1) for k in range(r)]
                    for (ca, cb, cd, xda, xdb, xdd, cr) in pieces:
                        junk = pjunk.tile([128, 1], f32, tag="junk")
                        nc.vector._custom_dve(
                            TENSOR_TENSOR_REDUCE,
                            out=_slice_blocks(dstb, cd, xdd, cr),
                            in0=_slice_blocks(x1b, ca, xda, cr),
                            in1=_slice_blocks(x2b, cb, xdb, cr),
                            s0=0.0, s1=float(cv), accum_out=junk[:],
                        )
                elif kind == "add":
                    rk, o0, j0, r = pl
                    nc.gpsimd.tensor_add(
                        out=outt[:, o0 * 64:(o0 + r) * 64],
                        in0=outt[:, o0 * 64:(o0 + r) * 64],
                        in1=tmps_flat[rk][:, j0 * 64:(j0 + r) * 64],
                    )
                else:  # store: ACT reorders blocked (o,s,c) -> s-major staging
                    o_lo, o_hi = pl
                    w = o_hi - o_lo
                    stg = pstg.tile([128, S, w * 32], f32, tag="stg")
                    for s in range(S):
                        nc.scalar.activation(
                            out=stg[:, s, :], in_=outv[:, s, o_lo:o_hi, :],
                            func=copyf,
                        )
                        nc.sync.dma_start(
                            out=y[s * 128:(s + 1) * 128, o_lo * 32:o_hi * 32],
                            in_=stg[:, s, :],
                        )
    nc.finalize()
    return nc


def _get_nc(triples, in_dim, out_dim, b_core, no_broadcast=False):
    a, b, o, c = triples
    key = hash((in_dim, out_dim, b_core, no_broadcast, tuple(a), tuple(b),
                tuple(o), tuple(np.asarray(c).tolist())))
    if key not in _nc_cache:
        ttr_chains, tmp_sizes, add_runs = _make_plan(a, b, o, c)
        _nc_cache[key] = _build_nc(ttr_chains, tmp_sizes, add_runs,
                                   in_dim, out_dim, b_core,
                                   no_broadcast=no_broadcast)
    return _nc_cache[key]


# ----------------------------------------------------------------- entry point
def kernel(x1, x2, cg_tilde, repids_in1, repids_in2, repids_out, out_dim):
    x1 = np.ascontiguousarray(np.asarray(x1, dtype=np.float32))
    x2 = np.ascontiguousarray(np.asarray(x2, dtype=np.float32))
    cg = np.asarray(cg_tilde, dtype=np.float32)
    r1 = np.asarray(repids_in1)
    r2 = np.asarray(repids_in2)
    ro = np.asarray(repids_out)
    odim = int(np.asarray(out_dim))

    B, in_dim = x1.shape
    triples = _extract_triples(r1, r2, ro, cg)
    usable = (
        triples is not None and B % N_CORES == 0
        and (B // N_CORES) == 256 and odim % 32 == 0 and in_dim % 32 == 0
    )
    if not usable:
        return _numpy_fallback(x1, x2, cg, r1, r2, ro, odim)

    from concourse.bass_utils import run_bass_kernel_spmd

    b_core = B // N_CORES
    nc = _get_nc(triples, in_dim, odim, b_core)

    in_maps = [
        {"x1": x1[i * b_core:(i + 1) * b_core],
         "x2": x2[i * b_core:(i + 1) * b_core]}
        for i in range(N_CORES)
    ]
    res = run_bass_kernel_spmd(nc, in_maps, core_ids=list(range(N_CORES)))
    out = np.empty((B, odim), dtype=np.float32)
    for i in range(N_CORES):
        out[i * b_core:(i + 1) * b_core] = res.results[i]["y"]
    return out
